# revision 28
# baseline (speedup 1.0000x reference)
"""Trainium2 Bass kernel for GQA attention with int8-quantized QK^T (8-core SPMD).

Per-core shard c of 8 (4 q heads + 1 kv head):
  q = x @ Wq.T -> [t, 256]; k,v = x @ Wk.T/Wv.T -> [t, 64]
  per-token-per-head int8 absmax quantization of q, k (exact emulation via
  magic-number round; dequant scales folded into the stored fp16 values)
  scoresT[t2, t1] = k.T @ q per head; p = exp(scoresT) (bf16, no max-sub)
  attT[hd, t1] = v_aug.T @ p with ones column -> row 64 = sumexp
  normalize (reciprocal at partition 64 + ones-matmul broadcast),
  AllGather heads across cores (per t1-block half), out_c = WoT_c.T @ att_full
  (o_proj column-sharded -> host concatenates; no AllReduce)

Perf structure vs v1:
  - x/Wqkv in fp16 with host-prepermuted [partition-major] layouts (4KB DMA lines)
  - GpSimd queue carries ONLY weight DMAs + AllGathers + att_full reads
    (partition_broadcast removed -> no collective-induced queue cascade)
  - phase B token tiles interleaved with attention blocks 0-1; o_proj of
    blocks 0-2 interleaved into attention blocks 2-3 (keeps PE continuously
    busy -> full 2.4 GHz p-state)
  - per a-step: both heads' score MMs emitted before both attV MMs so exp
    latency doesn't serialize the PE queue
"""

import numpy as np
import ml_dtypes
from contextlib import ExitStack

import concourse.bass as bass
import concourse.mybir as mybir
import concourse.tile as tile
from concourse import bacc
from concourse.bass import ts, ds
from concourse.masks import make_identity

NCORES = 8
P = 128
S = 2048          # tokens
D = 2048          # model dim
HD = 64           # head dim
NHL = 4           # q heads per core
JQ = NHL * HD     # 256 (q cols per core)
NQK = JQ + HD     # 320 (q + k cols, the quantized part)
NQKV = JQ + 2 * HD  # 384
TT = S // P       # 16 token tiles
DT = D // P       # 16 d tiles
NB = 4            # t1 blocks
BN = S // NB      # 512
MAGIC = 12582912.0  # 1.5 * 2**23: (x + MAGIC) - MAGIC == round-half-even(x)
SM = HD ** -0.5   # 0.125
F32 = mybir.dt.float32
F32R = mybir.dt.float32r
BF16 = mybir.dt.bfloat16
FP16 = mybir.dt.float16
AF = mybir.ActivationFunctionType
ALU = mybir.AluOpType


def build_nc(debug_taps=False):
    nc = bacc.Bacc(target_bir_lowering=False, debug=False, num_devices=NCORES)
    # host-prepermuted layouts: per-partition lines are contiguous
    xh = nc.declare_dram_parameter("xh", [P, TT, DT * P], FP16, isOutput=False)
    wqkv = nc.declare_dram_parameter("wqkv", [P, DT, NQKV], FP16, isOutput=False)
    woT = nc.declare_dram_parameter("woT", [P, DT, JQ], BF16, isOutput=False)
    tri = nc.declare_dram_parameter("tri", [P, P], BF16, isOutput=False)
    out_ext = nc.declare_dram_parameter("out", [JQ, S], F32, isOutput=True)
    taps = None
    if debug_taps:
        taps = {
            "qT_d": nc.declare_dram_parameter("qT_d", [HD, NHL, S], FP16, isOutput=True),
            "kT_d": nc.declare_dram_parameter("kT_d", [HD, S], FP16, isOutput=True),
            "v_d": nc.declare_dram_parameter("v_d", [P, TT, HD + 1], BF16, isOutput=True),
            "se_d": nc.declare_dram_parameter("se_d", [2, BN], F32, isOutput=True),
            "rcp_d": nc.declare_dram_parameter("rcp_d", [2, BN], F32, isOutput=True),
            "rbs_d": nc.declare_dram_parameter("rbs_d", [2 * HD, BN], F32, isOutput=True),
            "att_d": nc.declare_dram_parameter("att_d", [JQ, BN], BF16, isOutput=True),
        }
    with tile.TileContext(nc) as tc:
        with ExitStack() as ctx:
            _body(nc, tc, ctx, xh, wqkv, woT, tri, out_ext, taps)
    nc.finalize()
    return nc


def _body(nc, tc, ctx, xh, wqkv, woT, tri, out_ext, taps=None):
    dram_pool = ctx.enter_context(tc.tile_pool(name="dram", bufs=1, space="DRAM"))
    att_shard = [
        dram_pool.tile([JQ, BN], BF16, name=f"att_shard{b}", tag=f"as{b}")
        for b in range(NB)
    ]
    att_full = [
        [dram_pool.tile([NCORES * P, BN], BF16, addr_space="Shared",
                        name=f"att_full{b}_{pr}", tag=f"af{b}_{pr}")
         for pr in range(2)]
        for b in range(NB)
    ]

    singles = ctx.enter_context(tc.tile_pool(name="singles", bufs=1))
    xpool = ctx.enter_context(tc.tile_pool(name="xpool", bufs=3))
    quant = ctx.enter_context(tc.tile_pool(name="quant", bufs=3))
    p_pool = ctx.enter_context(tc.tile_pool(name="pp", bufs=4))
    bc_sb = ctx.enter_context(tc.tile_pool(name="bc_sb", bufs=3))
    an_sb = ctx.enter_context(tc.tile_pool(name="an_sb", bufs=3))
    orhs = ctx.enter_context(tc.tile_pool(name="orhs", bufs=34))
    osb = ctx.enter_context(tc.tile_pool(name="osb", bufs=2))
    # PSUM: 8 banks of 2KB/partition
    # main: score tiles + rbs broadcasts (2) | at: atps accumulators + phase-B
    # qkv tiles (4) | aux: phase-B transposes + o_proj accumulators (2)
    ps_main = ctx.enter_context(tc.tile_pool(name="ps_main", bufs=2, space="PSUM"))
    ps_at = ctx.enter_context(tc.tile_pool(name="ps_at", bufs=4, space="PSUM"))
    ps_aux = ctx.enter_context(tc.tile_pool(name="ps_aux", bufs=2, space="PSUM"))

    # ---------------- persistent tiles ----------------
    wqkv_sb = singles.tile([P, DT, NQKV], FP16)
    for c in range(4):
        nc.gpsimd.dma_start(out=wqkv_sb[:, 4 * c:4 * c + 4, :],
                            in_=wqkv[:, 4 * c:4 * c + 4, :])
    woT_sb = singles.tile([P, DT, JQ], BF16)
    nc.gpsimd.dma_start(out=woT_sb, in_=woT[:, :, :])
    tri_sb = singles.tile([P, P], BF16)
    nc.sync.dma_start(out=tri_sb, in_=tri[:, :])
    id_fp16 = singles.tile([P, P], FP16)
    make_identity(nc, id_fp16)
    qT_sb = singles.tile([HD, NHL, S], FP16)  # dequantized q: [hd, head, t]
    kT_sb = singles.tile([HD, S], FP16)       # dequantized k (incl sm)
    v_sb = singles.tile([P, TT, HD + 1], BF16)
    nc.vector.memset(v_sb, 1.0)             # col 64 stays 1.0 (sumexp trick)
    magic_sb = singles.tile([P, HD], F32)
    nc.vector.memset(magic_sb, MAGIC)
    ones_hi = singles.tile([HD + 1, HD], F32R)  # row 64 used as bcast lhsT
    ones_f32 = singles.tile([HD + 1, HD], F32)
    nc.vector.memset(ones_f32, 1.0)
    nc.vector.tensor_copy(ones_hi, ones_f32)

    # ---------------- phase B: one token tile ----------------
    qki_live = {}  # i -> qki tile awaiting transpose

    def emit_b_tile(i):
        xcol = xpool.tile([P, DT * P], FP16, tag="xcol")
        nc.sync.dma_start(out=xcol, in_=xh[:, i, :])
        xc = xcol.rearrange("p (a m) -> p a m", m=P)
        qkv = ps_at.tile([P, BN], F32, tag="at")
        for d in range(DT):
            nc.tensor.matmul(
                qkv[:, 0:NQKV], lhsT=xc[:, d, :], rhs=wqkv_sb[:, d, :],
                start=(d == 0), stop=(d == DT - 1))
        # transpose of previous tile's qki (lag 1 hides the quant latency)
        if i - 1 in qki_live:
            emit_transpose(i - 1)
        # v -> bf16 (ones column at 64 preset)
        nc.vector.tensor_copy(v_sb[:, i, 0:HD], qkv[:, NQK:NQKV])
        # absmax over each head group of 64 (q heads 0-3, k group 4)
        amax = quant.tile([P, 5], F32, tag="amax")
        nc.vector.tensor_reduce(
            amax, qkv[:, 0:NQK].rearrange("p (g h) -> p g h", h=HD),
            axis=mybir.AxisListType.X, op=ALU.max, apply_absolute_value=True)
        amax_c = quant.tile([P, 5], F32, tag="amaxc")
        nc.vector.tensor_scalar_max(amax_c, amax, 1e-6)
        rec = quant.tile([P, 5], F32, tag="rec")
        nc.vector.reciprocal(rec, amax_c)
        scl = quant.tile([P, 5], F32, tag="scl")
        nc.vector.tensor_scalar_mul(scl, rec, 127.0)
        deq5 = quant.tile([P, 5], F32, tag="deq5")
        nc.vector.tensor_scalar_mul(deq5[:, 0:NHL], amax_c[:, 0:NHL], 1.0 / 127.0)
        nc.vector.tensor_scalar_mul(deq5[:, 4:5], amax_c[:, 4:5], SM / 127.0)
        # round-to-int via magic number, all on DVE:
        # tmp = qkv*scl + MAGIC ; qki = (tmp - MAGIC) * deq -> fp16
        tmp = quant.tile([P, NQK], F32, tag="tmp")
        for h in range(5):
            nc.vector.scalar_tensor_tensor(
                out=tmp[:, ts(h, HD)], in0=qkv[:, ts(h, HD)],
                scalar=scl[:, h:h + 1], in1=magic_sb,
                op0=ALU.mult, op1=ALU.add)
        qki = quant.tile([P, NQK], FP16, tag="qki")
        for h in range(5):
            nc.vector.tensor_scalar(
                qki[:, ts(h, HD)], tmp[:, ts(h, HD)], -MAGIC,
                deq5[:, h:h + 1], ALU.add, ALU.mult)
        qki_live[i] = qki

    def emit_transpose(i):
        qki = qki_live.pop(i)
        for h in range(4):
            tp = ps_aux.tile([HD, P], FP16, tag="aux")
            nc.tensor.transpose(tp, qki[:, ts(h, HD)], id_fp16)
            if h < 2:
                nc.scalar.activation(
                    out=qT_sb[:, h, ts(i, P)], in_=tp, func=AF.Copy)
            else:
                nc.vector.tensor_copy(qT_sb[:, h, ts(i, P)], tp)
        tpk = ps_aux.tile([HD, P], FP16, tag="aux")
        nc.tensor.transpose(tpk, qki[:, JQ:NQK], id_fp16)
        nc.vector.tensor_copy(kT_sb[:, ts(i, P)], tpk)

    # ---------------- phase C: attention ----------------
    state = {}  # (b, pair) -> atps list
    pend_ops = []  # deferred o_proj thunks

    def drain(n):
        for _ in range(min(n, len(pend_ops))):
            pend_ops.pop(0)()

    def emit_c_step(b, pair, a):
        na = 4 * (b + 1)
        key = (b, pair)
        if key not in state:
            state[key] = [
                ps_at.tile([HD + 1, BN], F32, tag="at", name=f"at{b}_{pair}_{hh}")
                for hh in range(2)]
        atps = state[key]
        arel = a - 4 * b
        off = max(0, arel) * P
        n_sub = BN - off
        scs = []
        for hh in range(2):
            sc = ps_main.tile([P, BN], F32, tag="mm")
            nc.tensor.matmul(
                sc[:, off:], lhsT=kT_sb[:, ts(a, P)],
                rhs=qT_sb[:, 2 * pair + hh, ds(b * BN + off, n_sub)],
                start=True, stop=True)
            scs.append(sc)
        pts = []
        for hh in range(2):
            pt = p_pool.tile([P, BN], BF16, tag="pt")
            nc.scalar.activation(
                out=pt[:, off:], in_=scs[hh][:, off:], func=AF.Exp)
            if arel >= 0:
                nc.vector.tensor_mul(
                    pt[:, off:off + P], pt[:, off:off + P], tri_sb)
            pts.append(pt)
        for hh in range(2):
            nc.tensor.matmul(
                atps[hh][:, off:], lhsT=v_sb[:, a, :], rhs=pts[hh][:, off:],
                start=(a == 0), stop=(a == na - 1))
        drain(2)

    def emit_c_pair_finish(b, pair, defer=False):
        atps = state.pop((b, pair))
        se = bc_sb.tile([HD + 1, 2, BN], F32, tag="se")
        for hh in range(2):
            nc.vector.tensor_copy(se[HD:HD + 1, hh, :], atps[hh][HD:HD + 1, :])
        # partition 64 -> 0 hop: reciprocal_approx_fast microcode misbehaves
        # off partition 0
        se0 = bc_sb.tile([1, 2, BN], F32, tag="se0")
        for hh in range(2):
            nc.sync.dma_start(out=se0[0:1, hh, :], in_=se[HD:HD + 1, hh, :])
        rcp = bc_sb.tile([1, 2, BN], F32, tag="rcp")
        nc.vector.reciprocal_approx_fast(rcp[0:1, :, :], se0[0:1, :, :])
        rcpr = bc_sb.tile([1, 2, BN], F32R, tag="rcpr")
        nc.vector.tensor_copy(rcpr[0:1, :, :], rcp[0:1, :, :])
        if taps is not None and b == 0 and pair == 0:
            for hh in range(2):
                nc.sync.dma_start(out=taps["se_d"][hh:hh + 1, :],
                                  in_=se0[0:1, hh, :])
                nc.sync.dma_start(out=taps["rcp_d"][hh:hh + 1, :],
                                  in_=rcp[0:1, hh, :])

        def mk_rbs(hh):
            def f():
                h = 2 * pair + hh
                rbs = ps_main.tile([P, BN], F32, tag="mm")
                nc.tensor.matmul(
                    rbs[0:HD, :], lhsT=ones_hi[0:1, :],
                    rhs=rcpr[0:1, hh, :], start=True, stop=True)
                rbs_sb = an_sb.tile([HD, BN], F32, tag="rbs_sb")
                nc.vector.tensor_copy(rbs_sb, rbs[0:HD, :])
                ans = an_sb.tile([HD, BN], BF16, tag="ans")
                nc.vector.tensor_mul(ans, atps[hh][0:HD, :], rbs_sb)
                nc.sync.dma_start(out=att_shard[b][ts(h, HD), :], in_=ans)
                if taps is not None and b == 0 and pair == 0:
                    nc.sync.dma_start(out=taps["rbs_d"][ts(hh, HD), :],
                                      in_=rbs_sb)
            return f

        def mk_ag():
            def f():
                nc.gpsimd.collective_compute(
                    "AllGather", ALU.bypass,
                    replica_groups=[list(range(NCORES))],
                    ins=[att_shard[b][ts(pair, P), :]],
                    outs=[att_full[b][pair][:, :]])
            return f

        parts = [mk_rbs(0), mk_rbs(1), mk_ag()]
        if defer:
            # near-front insert: drains ~2 steps into the next pair, late
            # enough to cover the reciprocal chain latency but early enough
            # to free the atps PSUM banks before the pair after next
            pend_ops[4:4] = parts
        else:
            for f in parts:
                f()

    # ---------------- o_proj (deferred thunks; rt DMAs issued eagerly) -----
    def queue_oproj(b):
        oph = [ps_aux.tile([P, BN], F32, tag="aux", name=f"op{b}_{m}")
               for m in range(2)]
        rts = []
        for half in range(2):
            for t in range(NCORES):
                rt = orhs.tile([P, BN], BF16, tag="rt", name=f"rt{b}_{half}_{t}")
                nc.gpsimd.dma_start(out=rt, in_=att_full[b][half][ts(t, P), :])
                rts.append(rt)

        def mk_mm(half, t, m):
            def f():
                nc.tensor.matmul(
                    oph[m], lhsT=woT_sb[:, 2 * t + half, ts(m, P)],
                    rhs=rts[NCORES * half + t],
                    start=(half == 0 and t == 0),
                    stop=(half == 1 and t == NCORES - 1))
            return f

        def mk_out(m):
            def f():
                ot = osb.tile([P, BN], F32, tag="ot", name=f"ot{b}_{m}")
                nc.vector.tensor_copy(ot, oph[m])
                nc.sync.dma_start(out=out_ext[ts(m, P), ts(b, BN)], in_=ot)
            return f

        # accumulation order must be half-major so start/stop flags line up
        for half in range(2):
            for t in range(NCORES):
                for m in range(2):
                    pend_ops.append(mk_mm(half, t, m))
        pend_ops.append(mk_out(0))
        pend_ops.append(mk_out(1))

    # ---------------- schedule ----------------
    for i in range(4):
        emit_b_tile(i)
    # block 0 (8 a-steps) interleaved with tiles 4..7
    c0 = [(0, p, a) for p in range(2) for a in range(4)]
    for j, i in enumerate(range(4, 8)):
        emit_b_tile(i)
        for b, pp, a in c0[2 * j:2 * j + 2]:
            emit_c_step(b, pp, a)
            if pp == 0 and a == 3:
                emit_c_pair_finish(0, 0)
    emit_c_pair_finish(0, 1)
    # block 1 (16 a-steps) interleaved with tiles 8..15 (one-tile lag)
    c1 = [(1, p, a) for p in range(2) for a in range(8)]
    ci = 0
    for i in range(8, 16):
        emit_b_tile(i)
        if i >= 9:
            for b, pp, a in c1[ci:ci + 2]:
                emit_c_step(b, pp, a)
                if a == 7:
                    emit_c_pair_finish(1, pp)
            ci += 2
    emit_transpose(15)
    queue_oproj(0)
    for b, pp, a in c1[ci:]:
        emit_c_step(b, pp, a)
        if a == 7:
            emit_c_pair_finish(1, pp, defer=(pp == 1))
    # blocks 2 and 3, draining o_proj + deferred-finish thunks in emit_c_step
    for pp in range(2):
        for a in range(12):
            emit_c_step(2, pp, a)
        emit_c_pair_finish(2, pp, defer=True)
    queue_oproj(1)
    for pp in range(2):
        for a in range(16):
            emit_c_step(3, pp, a)
            if pp == 1 and a == 0:
                queue_oproj(2)
        emit_c_pair_finish(3, pp, defer=True)
    drain(len(pend_ops))
    queue_oproj(3)
    drain(len(pend_ops))
    if taps is not None:
        nc.sync.dma_start(out=taps["qT_d"][:, :, :], in_=qT_sb)
        nc.sync.dma_start(out=taps["kT_d"][:, :], in_=kT_sb)
        nc.sync.dma_start(out=taps["v_d"][:, :, :], in_=v_sb)
        nc.sync.dma_start(out=taps["att_d"][:, :], in_=att_shard[0][:, :])


# ---------------- host side ----------------

def prep_in_maps(x, Wq, Wk, Wv, Wo):
    bf = ml_dtypes.bfloat16
    x1 = np.asarray(x, dtype=np.float32).reshape(S, D)
    # xh[p, i, a*128+m] = x1[i*128+m, a*128+p]
    xh = np.ascontiguousarray(
        x1.reshape(TT, P, DT, P).transpose(3, 0, 2, 1)
        .reshape(P, TT, DT * P).astype(np.float16))
    tri_h = np.ascontiguousarray(
        (np.arange(P)[:, None] <= np.arange(P)[None, :]).astype(bf))
    in_maps = []
    for c in range(NCORES):
        wq = Wq[c * JQ:(c + 1) * JQ, :].T
        wk = Wk[c * HD:(c + 1) * HD, :].T
        wv = Wv[c * HD:(c + 1) * HD, :].T
        w = np.concatenate([wq, wk, wv], axis=1)  # [D, NQKV]
        wqkv_h = np.ascontiguousarray(
            w.reshape(DT, P, NQKV).transpose(1, 0, 2).astype(np.float16))
        wo = Wo[c * JQ:(c + 1) * JQ, :].T          # [D, JQ]
        woT_h = np.ascontiguousarray(
            wo.reshape(DT, P, JQ).transpose(1, 0, 2).astype(bf))
        in_maps.append({"xh": xh, "wqkv": wqkv_h, "woT": woT_h, "tri": tri_h})
    return in_maps


def unshard(results):
    out = np.empty((S, D), dtype=np.float32)
    for c in range(NCORES):
        out[:, c * JQ:(c + 1) * JQ] = results[c]["out"].T
    return out.reshape(1, S, D)


def kernel(x, Wq, Wk, Wv, Wo):
    from concourse.bass_utils import run_bass_kernel_spmd
    nc = build_nc()
    in_maps = prep_in_maps(x, Wq, Wk, Wv, Wo)
    res = run_bass_kernel_spmd(nc, in_maps, core_ids=list(range(NCORES)))
    return unshard(res.results)


# revision 34
# speedup vs baseline: 1.1368x; 1.1368x over previous
"""Trainium2 Bass kernel for GQA attention with int8-quantized QK^T (8-core SPMD).

Per-core shard c of 8 (4 q heads + 1 kv head):
  q = x @ Wq.T -> [t, 256]; k,v = x @ Wk.T/Wv.T -> [t, 64]
  per-token-per-head int8 absmax quantization of q, k (exact emulation via
  magic-number round; dequant scales folded into the stored fp16 values)
  scoresT[t2, t1] = k.T @ q per head; p = exp(scoresT) (bf16, no max-sub)
  attT[hd, t1] = v_aug.T @ p with ones column -> row 64 = sumexp
  normalize (reciprocal at partition 64 + ones-matmul broadcast),
  AllGather heads across cores (per t1-block half), out_c = WoT_c.T @ att_full
  (o_proj column-sharded -> host concatenates; no AllReduce)

Perf structure vs v1:
  - x/Wqkv in fp16 with host-prepermuted [partition-major] layouts (4KB DMA lines)
  - GpSimd queue carries ONLY weight DMAs + AllGathers + att_full reads
    (partition_broadcast removed -> no collective-induced queue cascade)
  - phase B token tiles interleaved with attention blocks 0-1; o_proj of
    blocks 0-2 interleaved into attention blocks 2-3 (keeps PE continuously
    busy -> full 2.4 GHz p-state)
  - per a-step: both heads' score MMs emitted before both attV MMs so exp
    latency doesn't serialize the PE queue
"""

import numpy as np
import ml_dtypes
from contextlib import ExitStack

import concourse.bass as bass
import concourse.mybir as mybir
import concourse.tile as tile
from concourse import bacc
from concourse.bass import ts, ds
from concourse.masks import make_identity

NCORES = 8
P = 128
S = 2048          # tokens
D = 2048          # model dim
HD = 64           # head dim
NHL = 4           # q heads per core
JQ = NHL * HD     # 256 (q cols per core)
NQK = JQ + HD     # 320 (q + k cols, the quantized part)
NQKV = JQ + 2 * HD  # 384
TT = S // P       # 16 token tiles
DT = D // P       # 16 d tiles
NB = 4            # t1 blocks
BN = S // NB      # 512
MAGIC = 12582912.0  # 1.5 * 2**23: (x + MAGIC) - MAGIC == round-half-even(x)
SM = HD ** -0.5   # 0.125
F32 = mybir.dt.float32
F32R = mybir.dt.float32r
BF16 = mybir.dt.bfloat16
FP16 = mybir.dt.float16
AF = mybir.ActivationFunctionType
ALU = mybir.AluOpType


def build_nc(debug_taps=False):
    nc = bacc.Bacc(target_bir_lowering=False, debug=False, num_devices=NCORES)
    # host-prepermuted layouts: per-partition lines are contiguous
    xh = nc.declare_dram_parameter("xh", [P, TT, DT * P], FP16, isOutput=False)
    wqkv = nc.declare_dram_parameter("wqkv", [P, DT, NQKV], FP16, isOutput=False)
    woT = nc.declare_dram_parameter("woT", [P, DT, JQ], BF16, isOutput=False)
    tri = nc.declare_dram_parameter("tri", [P, P], BF16, isOutput=False)
    out_ext = nc.declare_dram_parameter("out", [JQ, S], F32, isOutput=True)
    taps = None
    if debug_taps:
        taps = {
            "qT_d": nc.declare_dram_parameter("qT_d", [HD, NHL, S], FP16, isOutput=True),
            "kT_d": nc.declare_dram_parameter("kT_d", [HD, S], FP16, isOutput=True),
            "v_d": nc.declare_dram_parameter("v_d", [P, TT, HD + 1], BF16, isOutput=True),
            "se_d": nc.declare_dram_parameter("se_d", [2, BN], F32, isOutput=True),
            "rcp_d": nc.declare_dram_parameter("rcp_d", [2, BN], F32, isOutput=True),
            "rbs_d": nc.declare_dram_parameter("rbs_d", [2 * HD, BN], F32, isOutput=True),
            "att_d": nc.declare_dram_parameter("att_d", [JQ, BN], BF16, isOutput=True),
        }
    with tile.TileContext(nc) as tc:
        with ExitStack() as ctx:
            _body(nc, tc, ctx, xh, wqkv, woT, tri, out_ext, taps)
    nc.finalize()
    return nc


def _body(nc, tc, ctx, xh, wqkv, woT, tri, out_ext, taps=None):
    dram_pool = ctx.enter_context(tc.tile_pool(name="dram", bufs=1, space="DRAM"))
    att_shard = [
        dram_pool.tile([JQ, BN], BF16, name=f"att_shard{b}", tag=f"as{b}")
        for b in range(NB)
    ]
    att_full = [
        [dram_pool.tile([NCORES * P, BN], BF16, addr_space="Shared",
                        name=f"att_full{b}_{pr}", tag=f"af{b}_{pr}")
         for pr in range(2)]
        for b in range(NB)
    ]

    singles = ctx.enter_context(tc.tile_pool(name="singles", bufs=1))
    xpool = ctx.enter_context(tc.tile_pool(name="xpool", bufs=3))
    quant = ctx.enter_context(tc.tile_pool(name="quant", bufs=3))
    p_pool = ctx.enter_context(tc.tile_pool(name="pp", bufs=4))
    bc_sb = ctx.enter_context(tc.tile_pool(name="bc_sb", bufs=3))
    an_sb = ctx.enter_context(tc.tile_pool(name="an_sb", bufs=3))
    orhs = ctx.enter_context(tc.tile_pool(name="orhs", bufs=34))
    osb = ctx.enter_context(tc.tile_pool(name="osb", bufs=2))
    # PSUM: 8 banks of 2KB/partition
    # main: score tiles + rbs broadcasts (2) | at: atps accumulators + phase-B
    # qkv tiles (4) | aux: phase-B transposes + o_proj accumulators (2)
    ps_main = ctx.enter_context(tc.tile_pool(name="ps_main", bufs=2, space="PSUM"))
    ps_at = ctx.enter_context(tc.tile_pool(name="ps_at", bufs=4, space="PSUM"))
    ps_aux = ctx.enter_context(tc.tile_pool(name="ps_aux", bufs=2, space="PSUM"))

    # ---------------- persistent tiles ----------------
    wqkv_sb = singles.tile([P, DT, NQKV], FP16)
    for c in range(4):
        nc.gpsimd.dma_start(out=wqkv_sb[:, 4 * c:4 * c + 4, :],
                            in_=wqkv[:, 4 * c:4 * c + 4, :])
    woT_sb = singles.tile([P, DT, JQ], BF16)
    nc.gpsimd.dma_start(out=woT_sb, in_=woT[:, :, :])
    tri_sb = singles.tile([P, P], BF16)
    nc.sync.dma_start(out=tri_sb, in_=tri[:, :])
    id_fp16 = singles.tile([P, P], FP16)
    make_identity(nc, id_fp16)
    qT_sb = singles.tile([HD, NHL, S], FP16)  # dequantized q: [hd, head, t]
    kT_sb = singles.tile([HD, S], FP16)       # dequantized k (incl sm)
    v_sb = singles.tile([P, TT, HD + 1], BF16)
    nc.vector.memset(v_sb, 1.0)             # col 64 stays 1.0 (sumexp trick)
    magic_sb = singles.tile([P, HD], F32)
    nc.vector.memset(magic_sb, MAGIC)
    ones_hi = singles.tile([HD + 1, HD], F32R)  # row 64 used as bcast lhsT
    ones_f32 = singles.tile([HD + 1, HD], F32)
    nc.vector.memset(ones_f32, 1.0)
    nc.vector.tensor_copy(ones_hi, ones_f32)

    # ---------------- phase B: one token tile ----------------
    qki_live = {}  # i -> qki tile awaiting transpose

    def emit_b_tile(i):
        xcol = xpool.tile([P, DT * P], FP16, tag="xcol")
        nc.sync.dma_start(out=xcol, in_=xh[:, i, :])
        xc = xcol.rearrange("p (a m) -> p a m", m=P)
        qkv = ps_at.tile([P, BN], F32, tag="at")
        for d in range(DT):
            nc.tensor.matmul(
                qkv[:, 0:NQKV], lhsT=xc[:, d, :], rhs=wqkv_sb[:, d, :],
                start=(d == 0), stop=(d == DT - 1))
        # transpose of previous tile's qki (lag 1 hides the quant latency)
        if i - 1 in qki_live:
            emit_transpose(i - 1)
        # v -> bf16 (ones column at 64 preset)
        nc.vector.tensor_copy(v_sb[:, i, 0:HD], qkv[:, NQK:NQKV])
        # absmax over each head group of 64 (q heads 0-3, k group 4)
        amax = quant.tile([P, 5], F32, tag="amax")
        nc.vector.tensor_reduce(
            amax, qkv[:, 0:NQK].rearrange("p (g h) -> p g h", h=HD),
            axis=mybir.AxisListType.X, op=ALU.max, apply_absolute_value=True)
        amax_c = quant.tile([P, 5], F32, tag="amaxc")
        nc.vector.tensor_scalar_max(amax_c, amax, 1e-6)
        rec = quant.tile([P, 5], F32, tag="rec")
        nc.vector.reciprocal(rec, amax_c)
        scl = quant.tile([P, 5], F32, tag="scl")
        nc.vector.tensor_scalar_mul(scl, rec, 127.0)
        deq5 = quant.tile([P, 5], F32, tag="deq5")
        nc.vector.tensor_scalar_mul(deq5[:, 0:NHL], amax_c[:, 0:NHL], 1.0 / 127.0)
        nc.vector.tensor_scalar_mul(deq5[:, 4:5], amax_c[:, 4:5], SM / 127.0)
        # round-to-int via magic number, all on DVE:
        # tmp = qkv*scl + MAGIC ; qki = (tmp - MAGIC) * deq -> fp16
        tmp = quant.tile([P, NQK], F32, tag="tmp")
        for h in range(5):
            nc.vector.scalar_tensor_tensor(
                out=tmp[:, ts(h, HD)], in0=qkv[:, ts(h, HD)],
                scalar=scl[:, h:h + 1], in1=magic_sb,
                op0=ALU.mult, op1=ALU.add)
        qki = quant.tile([P, NQK], FP16, tag="qki")
        for h in range(5):
            nc.vector.tensor_scalar(
                qki[:, ts(h, HD)], tmp[:, ts(h, HD)], -MAGIC,
                deq5[:, h:h + 1], ALU.add, ALU.mult)
        qki_live[i] = qki

    def emit_transpose(i):
        qki = qki_live.pop(i)
        for h in range(4):
            tp = ps_aux.tile([HD, P], FP16, tag="aux")
            nc.tensor.transpose(tp, qki[:, ts(h, HD)], id_fp16)
            if h < 2:
                nc.scalar.activation(
                    out=qT_sb[:, h, ts(i, P)], in_=tp, func=AF.Copy)
            else:
                nc.vector.tensor_copy(qT_sb[:, h, ts(i, P)], tp)
        tpk = ps_aux.tile([HD, P], FP16, tag="aux")
        nc.tensor.transpose(tpk, qki[:, JQ:NQK], id_fp16)
        nc.vector.tensor_copy(kT_sb[:, ts(i, P)], tpk)

    # ---------------- phase C: attention ----------------
    state = {}  # (b, pair) -> atps list
    pend_ops = []  # deferred o_proj thunks

    def drain(n):
        for _ in range(min(n, len(pend_ops))):
            pend_ops.pop(0)()

    def emit_c_step(b, pair, a):
        na = 4 * (b + 1)
        key = (b, pair)
        if key not in state:
            state[key] = [
                ps_at.tile([HD + 1, BN], F32, tag="at", name=f"at{b}_{pair}_{hh}")
                for hh in range(2)]
        atps = state[key]
        arel = a - 4 * b
        off = max(0, arel) * P
        n_sub = BN - off
        scs = []
        for hh in range(2):
            sc = ps_main.tile([P, BN], F32, tag="mm")
            nc.tensor.matmul(
                sc[:, off:], lhsT=kT_sb[:, ts(a, P)],
                rhs=qT_sb[:, 2 * pair + hh, ds(b * BN + off, n_sub)],
                start=True, stop=True)
            scs.append(sc)
        pts = []
        for hh in range(2):
            pt = p_pool.tile([P, BN], BF16, tag="pt")
            nc.scalar.activation(
                out=pt[:, off:], in_=scs[hh][:, off:], func=AF.Exp)
            if arel >= 0:
                nc.vector.tensor_mul(
                    pt[:, off:off + P], pt[:, off:off + P], tri_sb)
            pts.append(pt)
        for hh in range(2):
            nc.tensor.matmul(
                atps[hh][:, off:], lhsT=v_sb[:, a, :], rhs=pts[hh][:, off:],
                start=(a == 0), stop=(a == na - 1))
        drain(2)

    def emit_c_pair_finish(b, pair, defer=False):
        atps = state.pop((b, pair))
        se = bc_sb.tile([HD + 1, 2, BN], F32, tag="se")
        for hh in range(2):
            nc.vector.tensor_copy(se[HD:HD + 1, hh, :], atps[hh][HD:HD + 1, :])
        # partition 64 -> 0 hop via DMA: reciprocal_approx_fast microcode
        # misbehaves off partition 0 (and DMA cannot read PSUM)
        se0 = bc_sb.tile([1, 2, BN], F32, tag="se0")
        for hh in range(2):
            nc.sync.dma_start(out=se0[0:1, hh, :], in_=se[HD:HD + 1, hh, :])
        rcp = bc_sb.tile([1, 2, BN], F32, tag="rcp")
        nc.vector.reciprocal_approx_fast(rcp[0:1, :, :], se0[0:1, :, :])
        rcpr = bc_sb.tile([1, 2, BN], F32R, tag="rcpr")
        nc.vector.tensor_copy(rcpr[0:1, :, :], rcp[0:1, :, :])
        if taps is not None and b == 0 and pair == 0:
            for hh_ in range(2):
                nc.sync.dma_start(out=taps["se_d"][hh_:hh_ + 1, :],
                                  in_=se0[0:1, hh_, :])
                nc.sync.dma_start(out=taps["rcp_d"][hh_:hh_ + 1, :],
                                  in_=rcp[0:1, hh_, :])

        def mk_rbs(hh):
            def f():
                h = 2 * pair + hh
                rbs = ps_main.tile([P, BN], F32, tag="mm")
                nc.tensor.matmul(
                    rbs[0:HD, :], lhsT=ones_hi[0:1, :],
                    rhs=rcpr[0:1, hh, :], start=True, stop=True)
                rbs_sb = an_sb.tile([HD, BN], F32, tag="rbs_sb")
                nc.vector.tensor_copy(rbs_sb, rbs[0:HD, :])
                ans = an_sb.tile([HD, BN], BF16, tag="ans")
                nc.vector.tensor_mul(ans, atps[hh][0:HD, :], rbs_sb)
                nc.sync.dma_start(out=att_shard[b][ts(h, HD), :], in_=ans)
                if taps is not None and b == 0 and pair == 0:
                    nc.sync.dma_start(out=taps["rbs_d"][ts(hh, HD), :],
                                      in_=rbs_sb)
            return f

        def mk_ag():
            def f():
                nc.gpsimd.collective_compute(
                    "AllGather", ALU.bypass,
                    replica_groups=[list(range(NCORES))],
                    ins=[att_shard[b][ts(pair, P), :]],
                    outs=[att_full[b][pair][:, :]])
            return f

        parts = [mk_rbs(0), mk_rbs(1), mk_ag()]
        if defer:
            # near-front insert: drains ~2 steps into the next pair, late
            # enough to cover the reciprocal chain latency but early enough
            # to free the atps PSUM banks before the pair after next
            pend_ops[4:4] = parts
        else:
            for f in parts:
                f()

    # ---------------- o_proj (deferred thunks; rt DMAs issued eagerly) -----
    def queue_oproj(b):
        oph = [ps_aux.tile([P, BN], F32, tag="aux", name=f"op{b}_{m}")
               for m in range(2)]
        rts = [None] * (2 * NCORES)

        def load_rt(j):
            half, t = divmod(j, NCORES)
            rt = orhs.tile([P, BN], BF16, tag="rt", name=f"rt{b}_{half}_{t}")
            nc.gpsimd.dma_start(out=rt, in_=att_full[b][half][ts(t, P), :])
            rts[j] = rt

        for j in range(4):
            load_rt(j)

        def mk_mm(half, t, m):
            def f():
                # prefetch the rt tile 4 steps ahead (spreads DMA writes,
                # avoids bursty SBUF port pressure against the PE streams)
                j = NCORES * half + t
                if m == 0 and j + 4 < 2 * NCORES:
                    load_rt(j + 4)
                nc.tensor.matmul(
                    oph[m], lhsT=woT_sb[:, 2 * t + half, ts(m, P)],
                    rhs=rts[j],
                    start=(half == 0 and t == 0),
                    stop=(half == 1 and t == NCORES - 1))
            return f

        def mk_out(m):
            def f():
                ot = osb.tile([P, BN], F32, tag="ot", name=f"ot{b}_{m}")
                nc.vector.tensor_copy(ot, oph[m])
                nc.sync.dma_start(out=out_ext[ts(m, P), ts(b, BN)], in_=ot)
            return f

        # accumulation order must be half-major so start/stop flags line up
        for half in range(2):
            for t in range(NCORES):
                for m in range(2):
                    pend_ops.append(mk_mm(half, t, m))
        pend_ops.append(mk_out(0))
        pend_ops.append(mk_out(1))

    # ---------------- schedule ----------------
    for i in range(4):
        emit_b_tile(i)
    # block 0 (8 a-steps) interleaved with tiles 4..7
    c0 = [(0, p, a) for p in range(2) for a in range(4)]
    for j, i in enumerate(range(4, 8)):
        emit_b_tile(i)
        for b, pp, a in c0[2 * j:2 * j + 2]:
            emit_c_step(b, pp, a)
            if pp == 0 and a == 3:
                emit_c_pair_finish(0, 0)
    emit_c_pair_finish(0, 1)
    # block 1 (16 a-steps) interleaved with tiles 8..15 (one-tile lag)
    c1 = [(1, p, a) for p in range(2) for a in range(8)]
    ci = 0
    for i in range(8, 16):
        emit_b_tile(i)
        if i >= 9:
            for b, pp, a in c1[ci:ci + 2]:
                emit_c_step(b, pp, a)
                if a == 7:
                    emit_c_pair_finish(1, pp)
            ci += 2
    emit_transpose(15)
    queue_oproj(0)
    for b, pp, a in c1[ci:]:
        emit_c_step(b, pp, a)
        if a == 7:
            emit_c_pair_finish(1, pp, defer=(pp == 1))
    # blocks 2 and 3, draining o_proj + deferred-finish thunks in emit_c_step
    for pp in range(2):
        for a in range(12):
            emit_c_step(2, pp, a)
        emit_c_pair_finish(2, pp, defer=True)
    queue_oproj(1)
    for pp in range(2):
        for a in range(16):
            emit_c_step(3, pp, a)
            if pp == 1 and a == 0:
                queue_oproj(2)
        emit_c_pair_finish(3, pp, defer=True)
    drain(len(pend_ops))
    queue_oproj(3)
    drain(len(pend_ops))
    if taps is not None:
        nc.sync.dma_start(out=taps["qT_d"][:, :, :], in_=qT_sb)
        nc.sync.dma_start(out=taps["kT_d"][:, :], in_=kT_sb)
        nc.sync.dma_start(out=taps["v_d"][:, :, :], in_=v_sb)
        nc.sync.dma_start(out=taps["att_d"][:, :], in_=att_shard[0][:, :])


# ---------------- host side ----------------

def prep_in_maps(x, Wq, Wk, Wv, Wo):
    bf = ml_dtypes.bfloat16
    x1 = np.asarray(x, dtype=np.float32).reshape(S, D)
    # xh[p, i, a*128+m] = x1[i*128+m, a*128+p]
    xh = np.ascontiguousarray(
        x1.reshape(TT, P, DT, P).transpose(3, 0, 2, 1)
        .reshape(P, TT, DT * P).astype(np.float16))
    tri_h = np.ascontiguousarray(
        (np.arange(P)[:, None] <= np.arange(P)[None, :]).astype(bf))
    in_maps = []
    for c in range(NCORES):
        wq = Wq[c * JQ:(c + 1) * JQ, :].T
        wk = Wk[c * HD:(c + 1) * HD, :].T
        wv = Wv[c * HD:(c + 1) * HD, :].T
        w = np.concatenate([wq, wk, wv], axis=1)  # [D, NQKV]
        wqkv_h = np.ascontiguousarray(
            w.reshape(DT, P, NQKV).transpose(1, 0, 2).astype(np.float16))
        wo = Wo[c * JQ:(c + 1) * JQ, :].T          # [D, JQ]
        woT_h = np.ascontiguousarray(
            wo.reshape(DT, P, JQ).transpose(1, 0, 2).astype(bf))
        in_maps.append({"xh": xh, "wqkv": wqkv_h, "woT": woT_h, "tri": tri_h})
    return in_maps


def unshard(results):
    out = np.empty((S, D), dtype=np.float32)
    for c in range(NCORES):
        out[:, c * JQ:(c + 1) * JQ] = results[c]["out"].T
    return out.reshape(1, S, D)


def kernel(x, Wq, Wk, Wv, Wo):
    from concourse.bass_utils import run_bass_kernel_spmd
    nc = build_nc()
    in_maps = prep_in_maps(x, Wq, Wk, Wv, Wo)
    res = run_bass_kernel_spmd(nc, in_maps, core_ids=list(range(NCORES)))
    return unshard(res.results)
